# revision 1
# baseline (speedup 1.0000x reference)
"""4-layer GAT (GATConv x4 + log_softmax) on 8 Trainium2 NeuronCores.

Strategy (node/row sharding + edge-parallel segment softmax):
  - Core c owns node rows [c*NPC, (c+1)*NPC).
  - Per layer l:
    Phase A: h = x @ Waug  row-sharded on each core's own nodes.
             Waug = [W | W@a_src | W@a_dst], so columns dout/dout+1 of the
             matmul output are the per-node scores ss/sd for free.
             Rows [h | ss | sd | 0pad] written to a local DRAM table.
    Phase B: AllGather the per-core tables -> full augmented table
             [N, pad] replicated on every core.
    Phase C: edge aggregation for the core's own dst nodes. Edges are
             bucketed by (core, dst-tile of 128 nodes) on the host and
             padded to C chunks of 128 edges. Per chunk:
               - indirect-DMA gather of the 128 source rows (h|ss)
               - S01[p,q] = (dstloc[p] == q)  one-hot dst matrix (DVE)
               - sd_edge = rowsum(S01 * sdbc)  (per-edge dst score)
               - e = ss + sd_edge  (attention logit per edge)
             Per group of G chunks: w = exp(leakyrelu(e)) (batched ACT).
             Per chunk: S = S01 * w (per-partition scalar), then PE
             matmuls accumulate  psum[dst, :] += S^T @ [h rows | 1]
             giving both the weighted message sum and (via a ones rhs
             column) the softmax denominator.  out = num/den (+relu, or
             log_softmax for the last layer).
  - Softmax max-subtraction is skipped: logits are O(+-10) for this
    model family so exp() cannot overflow in fp32; the result is
    mathematically identical to the max-subtracted form.
"""

import numpy as np

import concourse.bass as bass
import concourse.bacc as bacc
import concourse.mybir as mybir
import concourse.tile as tile
from concourse import bass_utils
from concourse.masks import make_identity

NCORES = 8
P = 128
NEG_SLOPE = 0.2
EPS = 1e-16
F32 = mybir.dt.float32
I32 = mybir.dt.int32


def _pad_to(n, mult):
    return ((n + mult - 1) // mult) * mult


class Cfg:
    def __init__(self, N, dims, C):
        assert N % NCORES == 0
        self.N = N
        self.dims = dims                      # [(din, dout), ...]
        self.C = C                            # chunks per dst tile
        self.NPC = N // NCORES                # nodes per core
        self.NT = (self.NPC + P - 1) // P     # dst tiles per core
        self.nlast = self.NPC - (self.NT - 1) * P
        # padded row width of the augmented table: >= dout+2, 16-elem aligned
        self.pads = [_pad_to(dout + 2, 16) for _, dout in dims]
        self.xtpad = _pad_to(self.NPC, P)     # padded node column count


def col_splits(width):
    """Split [0,width) into PSUM-bank-sized matmul column regions (<=512)."""
    out = []
    c = 0
    while c < width:
        out.append((c, min(c + 512, width)))
        c += 512
    return out


def build_program(cfg: Cfg, group=8, g_bufs=12, xt_bufs=None, dbg=False):
    nl = len(cfg.dims)
    nc = bacc.Bacc("TRN2", num_devices=NCORES)

    # ---- external inputs ----
    din0 = cfg.dims[0][0]
    xT_d = nc.dram_tensor("xT", [din0, cfg.xtpad], F32, kind="ExternalInput")
    W_d = [
        nc.dram_tensor(f"W{l}", [cfg.dims[l][0], cfg.pads[l]], F32, kind="ExternalInput")
        for l in range(nl)
    ]
    idx_d = nc.dram_tensor("idx", [cfg.NT, P, cfg.C], I32, kind="ExternalInput")
    dl_d = nc.dram_tensor("dl", [cfg.NT, P, cfg.C], F32, kind="ExternalInput")
    iota_d = nc.dram_tensor("iota", [P, P], F32, kind="ExternalInput")
    dlast = cfg.dims[-1][1]
    out_d = nc.dram_tensor("out", [cfg.NPC, dlast], F32, kind="ExternalOutput")
    if dbg:
        dbg_haug = nc.dram_tensor("dbg_haug", [cfg.N, cfg.pads[0]], F32,
                                  kind="ExternalOutput")
        dbg_eall = nc.dram_tensor("dbg_eall", [P, cfg.C], F32, kind="ExternalOutput")
        dbg_wall = nc.dram_tensor("dbg_wall", [P, cfg.C], F32, kind="ExternalOutput")
        dbg_num = nc.dram_tensor("dbg_num", [P, cfg.pads[0]], F32, kind="ExternalOutput")

    maxpad = max(cfg.pads)
    maxdout = max(dout for _, dout in cfg.dims)
    maxkc = max(d // P for d, _ in cfg.dims)
    if xt_bufs is None:
        # enough slots for two consecutive layers' lhsT chunks
        xt_bufs = min(12, 2 * maxkc)

    with tile.TileContext(nc) as tc:
        with (
            tc.tile_pool(name="xt", bufs=xt_bufs) as xt_pool,
            tc.tile_pool(name="w", bufs=maxkc + 1) as w_pool,
            tc.tile_pool(name="g", bufs=g_bufs) as g_pool,
            tc.tile_pool(name="stg", bufs=3) as stg_pool,
            tc.tile_pool(name="s01", bufs=group + 3) as s01_pool,
            tc.tile_pool(name="tmp", bufs=3) as tmp_pool,
            tc.tile_pool(name="small", bufs=4) as small_pool,
            tc.tile_pool(name="sdcol", bufs=2 * cfg.NT + 2) as sdcol_pool,
            tc.tile_pool(name="consts", bufs=1) as const_pool,
            tc.tile_pool(name="acc", bufs=2, space="PSUM") as acc_pool,
            tc.tile_pool(name="aux", bufs=2, space="PSUM") as aux_pool,
            tc.tile_pool(name="tpose", bufs=2, space="PSUM") as tp_pool,
            tc.tile_pool(name="dram", bufs=1, space="DRAM") as dram_pool,
        ):
            ident = const_pool.tile([P, P], F32, tag="ident")
            make_identity(nc, ident[:])
            iota_sb = const_pool.tile([P, P], F32, tag="iota")
            nc.sync.dma_start(out=iota_sb[:], in_=iota_d[:, :])
            ones_sb = const_pool.tile([P, 16], F32, tag="ones")
            nc.vector.memset(ones_sb[:], 1.0)

            # lhsT chunks of the current layer input ([P, xtpad] each)
            xt_cur = []
            for k in range(cfg.dims[0][0] // P):
                t = xt_pool.tile([P, cfg.xtpad], F32, tag="xt")
                nc.sync.dma_start(out=t[:], in_=xT_d[k * P:(k + 1) * P, :])
                xt_cur.append(t)

            for l in range(nl):
                din, dout = cfg.dims[l]
                pad = cfg.pads[l]
                kc = din // P
                last = l == nl - 1

                # ---- phase A: h = x @ Waug on own rows ----
                w_tiles = []
                for k in range(kc):
                    t = w_pool.tile([P, maxpad], F32, tag="w")
                    nc.sync.dma_start(out=t[:, 0:pad], in_=W_d[l][k * P:(k + 1) * P, :])
                    w_tiles.append(t)

                ag_in = dram_pool.tile([cfg.NPC, pad], F32, tag=f"agin{l}")
                ag_out = dram_pool.tile([cfg.N, pad], F32, tag=f"agout{l}",
                                        addr_space="Shared")

                sd_cols = []
                for m in range(cfg.NT):
                    rows = P if m < cfg.NT - 1 else cfg.nlast
                    ph = acc_pool.tile([P, maxdout], F32, space="PSUM", tag="acc")
                    px = aux_pool.tile([P, 16], F32, space="PSUM", tag="aux")
                    for k in range(kc):
                        for (a, b) in col_splits(dout):
                            nc.tensor.matmul(
                                out=ph[:, a:b],
                                lhsT=xt_cur[k][:, m * P:(m + 1) * P],
                                rhs=w_tiles[k][:, a:b],
                                start=(k == 0),
                                stop=(k == kc - 1),
                            )
                        nc.tensor.matmul(
                            out=px[:, 0:pad - dout],
                            lhsT=xt_cur[k][:, m * P:(m + 1) * P],
                            rhs=w_tiles[k][:, dout:pad],
                            start=(k == 0),
                            stop=(k == kc - 1),
                        )
                    stg = stg_pool.tile([P, maxpad], F32, tag="stg")
                    nc.vector.tensor_copy(out=stg[:, 0:dout], in_=ph[:, 0:dout])
                    nc.vector.tensor_copy(out=stg[:, dout:pad], in_=px[:, 0:pad - dout])
                    sdc = sdcol_pool.tile([P, 1], F32, tag="sdc")
                    nc.vector.tensor_copy(out=sdc[:], in_=stg[:, dout + 1:dout + 2])
                    sd_cols.append(sdc)
                    nc.sync.dma_start(
                        out=ag_in[m * P:m * P + rows, :],
                        in_=stg[0:rows, 0:pad],
                    )

                # ---- phase B: all-gather the augmented table ----
                nc.gpsimd.collective_compute(
                    "AllGather",
                    mybir.AluOpType.bypass,
                    replica_groups=[list(range(NCORES))],
                    ins=[ag_in[:, :].opt()],
                    outs=[ag_out[:, :].opt()],
                )

                if dbg and l == 0:
                    nc.gpsimd.dma_start(out=dbg_haug[:, :], in_=ag_out[:, :])

                # ---- phase C: edge aggregation for own dst nodes ----
                xt_next = []
                if not last:
                    for k in range(dout // P):
                        xt_next.append(xt_pool.tile([P, cfg.xtpad], F32, tag="xt", name=f"xtn{l}_{k}"))

                for m in range(cfg.NT):
                    rows = P if m < cfg.NT - 1 else cfg.nlast
                    idx_sb = small_pool.tile([P, cfg.C], I32, tag="idx")
                    nc.sync.dma_start(out=idx_sb[:], in_=idx_d[m, :, :])
                    dl_sb = small_pool.tile([P, cfg.C], F32, tag="dl")
                    nc.sync.dma_start(out=dl_sb[:], in_=dl_d[m, :, :])

                    # sdbc[p, q] = sd of local dst q (materialized via PE transpose)
                    ptp = tp_pool.tile([P, P], F32, space="PSUM", tag="tp")
                    nc.tensor.transpose(
                        out=ptp[:], in_=sd_cols[m][:, 0:1].to_broadcast([P, P]),
                        identity=ident[:],
                    )
                    sdbc = small_pool.tile([P, P], F32, tag="sdbc")
                    nc.vector.tensor_copy(out=sdbc[:], in_=ptp[:])

                    e_all = small_pool.tile([P, cfg.C], F32, tag="eall")
                    w_all = small_pool.tile([P, cfg.C], F32, tag="wall")

                    po = acc_pool.tile([P, maxdout], F32, space="PSUM", tag="acc")
                    pd = aux_pool.tile([P, 16], F32, space="PSUM", tag="aux")

                    gs, s01s = [], []
                    for j in range(cfg.C):
                        g = g_pool.tile([P, maxpad], F32, tag="g")
                        nc.gpsimd.indirect_dma_start(
                            out=g[:, 0:pad],
                            out_offset=None,
                            in_=ag_out[:, :],
                            in_offset=bass.IndirectOffsetOnAxis(
                                ap=idx_sb[:, j:j + 1], axis=0,
                            ),
                        )
                        s01 = s01_pool.tile([P, P], F32, tag="s01")
                        nc.vector.tensor_tensor(
                            out=s01[:],
                            in0=dl_sb[:, j:j + 1].to_broadcast([P, P]),
                            in1=iota_sb[:],
                            op=mybir.AluOpType.is_equal,
                        )
                        tmp = tmp_pool.tile([P, P], F32, tag="tmq")
                        nc.vector.tensor_mul(out=tmp[:], in0=s01[:], in1=sdbc[:])
                        sde = small_pool.tile([P, 1], F32, tag="sde")
                        nc.vector.reduce_sum(out=sde[:], in_=tmp[:],
                                             axis=mybir.AxisListType.X)
                        nc.vector.tensor_add(
                            out=e_all[:, j:j + 1],
                            in0=g[:, dout:dout + 1],
                            in1=sde[:],
                        )
                        gs.append(g)
                        s01s.append(s01)

                        if j % group == group - 1 or j == cfg.C - 1:
                            g0 = (j // group) * group
                            # w = exp(leakyrelu(e)) batched over the group
                            sl = slice(g0, j + 1)
                            nc.vector.tensor_scalar_mul(
                                out=w_all[:, sl], in0=e_all[:, sl],
                                scalar1=NEG_SLOPE,
                            )
                            nc.vector.tensor_tensor(
                                out=w_all[:, sl], in0=w_all[:, sl],
                                in1=e_all[:, sl], op=mybir.AluOpType.max,
                            )
                            nc.scalar.activation(
                                out=w_all[:, sl], in_=w_all[:, sl],
                                func=mybir.ActivationFunctionType.Exp,
                            )
                            for jj in range(g0, j + 1):
                                s = tmp_pool.tile([P, P], F32, tag="s")
                                nc.vector.tensor_scalar_mul(
                                    out=s[:], in0=s01s[jj][:],
                                    scalar1=w_all[:, jj:jj + 1],
                                )
                                for (a, b) in col_splits(dout):
                                    nc.tensor.matmul(
                                        out=po[:, a:b], lhsT=s[:],
                                        rhs=gs[jj][:, a:b],
                                        start=(jj == 0), stop=(jj == cfg.C - 1),
                                    )
                                nc.tensor.matmul(
                                    out=pd[:, 0:1], lhsT=s[:],
                                    rhs=ones_sb[:, 0:1],
                                    start=(jj == 0), stop=(jj == cfg.C - 1),
                                )

                    if dbg and l == 0 and m == 0:
                        nc.sync.dma_start(out=dbg_eall[:, :], in_=e_all[:])
                        nc.sync.dma_start(out=dbg_wall[:, :], in_=w_all[:])
                        dnum = stg_pool.tile([P, maxpad], F32, tag="stg")
                        nc.vector.tensor_copy(out=dnum[:, 0:dout], in_=po[:, 0:dout])
                        nc.vector.tensor_copy(out=dnum[:, dout:dout + 1], in_=pd[:, 0:1])
                        nc.sync.dma_start(out=dbg_num[:, :], in_=dnum[:, 0:pad])

                    # normalize: rec = 1/(den+eps)
                    dtmp = small_pool.tile([P, 1], F32, tag="dtmp")
                    nc.vector.tensor_scalar_add(
                        out=dtmp[:], in0=pd[:, 0:1], scalar1=EPS)
                    rec = small_pool.tile([P, 1], F32, tag="rec")
                    nc.vector.reciprocal(out=rec[:], in_=dtmp[:])

                    if not last:
                        relu_t = stg_pool.tile([P, maxpad], F32, tag="stg")
                        # fused (num * rec) max 0  (bias is zero)
                        nc.vector.tensor_scalar(
                            out=relu_t[:, 0:dout], in0=po[:, 0:dout],
                            scalar1=rec[:, 0:1], scalar2=0.0,
                            op0=mybir.AluOpType.mult, op1=mybir.AluOpType.max,
                        )
                        for k in range(dout // P):
                            ptt = tp_pool.tile([P, P], F32, space="PSUM", tag="tp")
                            nc.tensor.transpose(
                                out=ptt[:], in_=relu_t[:, k * P:(k + 1) * P],
                                identity=ident[:],
                            )
                            nc.vector.tensor_copy(
                                out=xt_next[k][:, m * P:(m + 1) * P], in_=ptt[:])
                    else:
                        # log_softmax over features
                        t1 = stg_pool.tile([P, maxpad], F32, tag="stg")
                        nc.vector.tensor_scalar_mul(
                            out=t1[:, 0:dout], in0=po[:, 0:dout], scalar1=rec[:, 0:1])
                        mx = small_pool.tile([P, 1], F32, tag="mx")
                        nc.vector.reduce_max(out=mx[:], in_=t1[:, 0:dout],
                                             axis=mybir.AxisListType.X)
                        nc.vector.tensor_scalar_sub(
                            out=t1[:, 0:dout], in0=t1[:, 0:dout], scalar1=mx[:, 0:1])
                        ex = small_pool.tile([P, dlast], F32, tag="ex")
                        sm = small_pool.tile([P, 1], F32, tag="sm")
                        nc.scalar.activation(
                            out=ex[:], in_=t1[:, 0:dout],
                            func=mybir.ActivationFunctionType.Exp,
                            accum_out=sm[:])
                        lg = small_pool.tile([P, 1], F32, tag="lg")
                        nc.scalar.activation(
                            out=lg[:], in_=sm[:],
                            func=mybir.ActivationFunctionType.Ln)
                        nc.vector.tensor_scalar_sub(
                            out=t1[:, 0:dout], in0=t1[:, 0:dout], scalar1=lg[:, 0:1])
                        nc.sync.dma_start(
                            out=out_d[m * P:m * P + rows, :],
                            in_=t1[0:rows, 0:dout],
                        )

                xt_cur = xt_next

    nc.compile()
    return nc


def prep_host(x, edge_index, Ws, a_srcs, a_dsts, cfg: Cfg):
    """Build per-core input maps."""
    N = cfg.N
    nl = len(cfg.dims)
    src = np.concatenate([np.asarray(edge_index[0]), np.arange(N, dtype=np.int64)])
    dst = np.concatenate([np.asarray(edge_index[1]), np.arange(N, dtype=np.int64)])
    src = src.astype(np.int64)
    dst = dst.astype(np.int64)

    c_of = dst // cfg.NPC
    r = dst - c_of * cfg.NPC
    t_of = r // P
    q = r - t_of * P
    key = c_of * cfg.NT + t_of
    order = np.argsort(key, kind="stable")
    counts = np.bincount(key, minlength=NCORES * cfg.NT)
    Cneed = int(np.ceil(counts.max() / P))
    assert Cneed <= cfg.C, f"need C>={Cneed}, got {cfg.C}"

    idx_a = np.zeros((NCORES, cfg.NT, P, cfg.C), dtype=np.int32)
    dl_a = np.full((NCORES, cfg.NT, P, cfg.C), -1.0, dtype=np.float32)
    starts = np.zeros(NCORES * cfg.NT + 1, dtype=np.int64)
    np.cumsum(counts, out=starts[1:])
    for g in range(NCORES * cfg.NT):
        seg = order[starts[g]:starts[g + 1]]
        if len(seg) == 0:
            continue
        c, t = divmod(g, cfg.NT)
        k = np.arange(len(seg))
        jj = k // P
        pp = k % P
        idx_a[c, t, pp, jj] = src[seg]
        dl_a[c, t, pp, jj] = q[seg]

    # augmented weights
    Waug = []
    for l in range(nl):
        W = np.asarray(Ws[l], dtype=np.float32)
        was = W @ np.asarray(a_srcs[l], dtype=np.float32)
        wad = W @ np.asarray(a_dsts[l], dtype=np.float32)
        A = np.zeros((W.shape[0], cfg.pads[l]), dtype=np.float32)
        A[:, :W.shape[1]] = W
        A[:, W.shape[1]] = was
        A[:, W.shape[1] + 1] = wad
        Waug.append(A)

    iota = np.tile(np.arange(P, dtype=np.float32), (P, 1))

    x = np.asarray(x, dtype=np.float32)
    in_maps = []
    for c in range(NCORES):
        xs = x[c * cfg.NPC:(c + 1) * cfg.NPC]          # [NPC, din0]
        xT = np.zeros((cfg.dims[0][0], cfg.xtpad), dtype=np.float32)
        xT[:, :cfg.NPC] = xs.T
        m = {
            "xT": np.ascontiguousarray(xT),
            "idx": np.ascontiguousarray(idx_a[c]),
            "dl": np.ascontiguousarray(dl_a[c]),
            "iota": iota,
        }
        for l in range(nl):
            m[f"W{l}"] = Waug[l]
        in_maps.append(m)
    return in_maps


def run(x, edge_index, Ws, a_srcs, a_dsts, cfg: Cfg, trace=False):
    # NOTE: idx/dl are shared across layers (same graph): device tensor is
    # [nl, NT, P, C] with identical content per layer to keep the program
    # uniform; built once here.
    in_maps = prep_host(x, edge_index, Ws, a_srcs, a_dsts, cfg)
    nc = build_program(cfg)
    res = bass_utils.run_bass_kernel_spmd(
        nc, in_maps, core_ids=list(range(NCORES)), trace=trace)
    out = np.concatenate([res.results[c]["out"][:cfg.NPC] for c in range(NCORES)],
                         axis=0)
    return out, res


FULL_CFG_DIMS = [(256, 1024), (1024, 1024), (1024, 512), (512, 128)]


def _full_cfg(edge_index):
    N = 10000
    # compute needed C from the data
    dst = np.concatenate([np.asarray(edge_index[1]), np.arange(N, dtype=np.int64)])
    npc = N // NCORES
    nt = (npc + P - 1) // P
    c_of = dst // npc
    r = dst - c_of * npc
    t_of = r // P
    counts = np.bincount(c_of * nt + t_of, minlength=NCORES * nt)
    C = int(np.ceil(counts.max() / P))
    return Cfg(N, FULL_CFG_DIMS, C)


def kernel(x, edge_index, W1, as1, ad1, b1, W2, as2, ad2, b2,
           W3, as3, ad3, b3, W4, as4, ad4, b4):
    for b in (b1, b2, b3, b4):
        assert not np.any(np.asarray(b)), "nonzero bias not implemented"
    cfg = _full_cfg(edge_index)
    out, _ = run(
        x, edge_index,
        [W1, W2, W3, W4], [as1, as2, as3, as4], [ad1, ad2, ad3, ad4], cfg)
    return out.astype(np.float32)



# revision 12
# speedup vs baseline: 1.9369x; 1.9369x over previous
"""4-layer GAT (GATConv x4 + log_softmax) on 8 Trainium2 NeuronCores.

Strategy (node/row sharding + edge-parallel segment softmax), v2 (bf16):
  - Core c owns node rows [c*NPC, (c+1)*NPC).
  - Per layer l (table row layout [h | ss | 1 | pad], bf16, width padw
    a multiple of 128 so the Ant dma_gather's 256B-granularity holds):
    Phase A: psum = x @ Waug row-sharded (bf16 lhsT/rhs, fp32 psum);
             Waug = [W | W@a_src | W@a_dst] so the per-node scores
             ss/sd are extra matmul columns. Rows are written (bf16) to
             ag_in; sd is broadcast to a local [NPC, 128] sd table
             (dst scores are only ever needed by the owning core).
    Phase B: AllGather ag_in -> full table ag_out [N, padw] bf16.
             Doubles as the cross-core barrier.
    Phase C: per dst tile (128 nodes), all C chunks of 128 edges are
             fetched with TWO Ant dma_gathers (one instruction each):
             rows of h[src] ([128, C, padw]) and sd[dst] ([128, C, 128]).
             e = ss_src + sd_dst; w = exp(leakyrelu(e)) batched per
             tile.  Per chunk one fused DVE op builds
             S = (iota == dl) * w in bf16, then PE matmuls accumulate
             psum[dst, :] += S^T @ [h | ss | 1] giving the weighted
             message sum and (ones column) the softmax denominator.
             out = num/den (+relu, or log_softmax for the last layer).
  - Softmax max-subtraction is skipped: logits are O(+-10) for this
    model family so exp() cannot overflow in fp32.
"""

import numpy as np
import ml_dtypes

import concourse.bass as bass
import concourse.bacc as bacc
import concourse.mybir as mybir
import concourse.tile as tile
from concourse import bass_utils
from concourse.masks import make_identity

NCORES = 8
P = 128
NEG_SLOPE = 0.2
EPS = 1e-16
F32 = mybir.dt.float32
BF16 = mybir.dt.bfloat16
I16 = mybir.dt.int16
BF = ml_dtypes.bfloat16


def _pad_to(n, mult):
    return ((n + mult - 1) // mult) * mult


class Cfg:
    def __init__(self, N, dims, C):
        assert N % NCORES == 0
        self.N = N
        self.dims = dims                      # [(din, dout), ...]
        self.C = C                            # chunks per dst tile
        self.NPC = N // NCORES                # nodes per core
        self.NT = (self.NPC + P - 1) // P     # dst tiles per core
        self.nlast = self.NPC - (self.NT - 1) * P
        # table row width: dout + 2 (ss, one), padded to 128 elems
        self.pads = [_pad_to(dout + 2, P) for _, dout in dims]
        self.xtpad = _pad_to(self.NPC, P)     # padded node column count


def col_splits(used):
    """Matmul column regions over [0, used): PSUM-bank (512 f32) chunks."""
    out = []
    c = 0
    while c < used:
        out.append((c, min(c + 512, used)))
        c += 512
    return out


def build_program(cfg: Cfg):
    nl = len(cfg.dims)
    nc = bacc.Bacc("TRN2", num_devices=NCORES, num_swdge_queues=4)

    din0 = cfg.dims[0][0]
    maxpad = max(cfg.pads)
    maxdout = max(dout for _, dout in cfg.dims)
    maxkc = max(d // P for d, _ in cfg.dims)
    dlast = cfg.dims[-1][1]
    CW = cfg.C * P // 16                      # wrapped idx columns

    xT_d = nc.dram_tensor("xT", [din0, cfg.xtpad], BF16, kind="ExternalInput")
    W_d = [
        nc.dram_tensor(f"W{l}", [cfg.dims[l][0], cfg.pads[l]], BF16,
                       kind="ExternalInput")
        for l in range(nl)
    ]
    hidx_d = nc.dram_tensor("hidx", [cfg.NT, P, CW], I16, kind="ExternalInput")
    sdidx_d = nc.dram_tensor("sdidx", [cfg.NT, P, CW], I16, kind="ExternalInput")
    dl_d = nc.dram_tensor("dl", [cfg.NT, P, cfg.C], F32, kind="ExternalInput")
    iota_d = nc.dram_tensor("iota", [P, P], F32, kind="ExternalInput")
    out_d = nc.dram_tensor("out", [cfg.NPC, dlast], F32, kind="ExternalOutput")

    with tile.TileContext(nc) as tc:
        with (
            tc.tile_pool(name="xt", bufs=2 * maxkc) as xt_pool,
            tc.tile_pool(name="w", bufs=maxkc + 1) as w_pool,
            tc.tile_pool(name="g", bufs=2) as g_pool,
            tc.tile_pool(name="sdg", bufs=2) as sdg_pool,
            tc.tile_pool(name="stg", bufs=3) as stg_pool,
            tc.tile_pool(name="s", bufs=6) as s_pool,
            tc.tile_pool(name="small", bufs=6) as small_pool,
            tc.tile_pool(name="consts", bufs=1) as const_pool,
            tc.tile_pool(name="acc", bufs=2, space="PSUM") as acc_pool,
            tc.tile_pool(name="tpose", bufs=2, space="PSUM") as tp_pool,
            tc.tile_pool(name="dram", bufs=1, space="DRAM") as dram_pool,
        ):
            identb = const_pool.tile([P, P], BF16, tag="identb")
            make_identity(nc, identb[:])
            iota_sb = const_pool.tile([P, P], F32, tag="iota")
            nc.sync.dma_start(out=iota_sb[:], in_=iota_d[:, :])

            # static per-tile edge metadata, loaded once
            hidx_sb, sdidx_sb, dl_sb = [], [], []
            for m in range(cfg.NT):
                t = const_pool.tile([P, CW], I16, tag=f"hidx{m}", name=f"hidx{m}")
                nc.sync.dma_start(out=t[:], in_=hidx_d[m, :, :])
                hidx_sb.append(t)
                t = const_pool.tile([P, CW], I16, tag=f"sdidx{m}", name=f"sdidx{m}")
                nc.sync.dma_start(out=t[:], in_=sdidx_d[m, :, :])
                sdidx_sb.append(t)
                t = const_pool.tile([P, cfg.C], F32, tag=f"dl{m}", name=f"dl{m}")
                nc.sync.dma_start(out=t[:], in_=dl_d[m, :, :])
                dl_sb.append(t)

            # per-layer DRAM scratch (exact widths keep every AP contiguous)
            ag_in = [dram_pool.tile([cfg.NPC, cfg.pads[l]], BF16, tag=f"agin{l}", name=f"agin{l}")
                     for l in range(nl)]
            ag_out = [dram_pool.tile([cfg.N, cfg.pads[l]], BF16, tag=f"agout{l}", name=f"agout{l}",
                                     addr_space="Shared") for l in range(nl)]
            sdtab = [dram_pool.tile([cfg.xtpad, P], BF16, tag=f"sdtab{l}", name=f"sdtab{l}")
                     for l in range(nl)]

            self_qn = [0]  # SWDGE queue round-robin counter

            # lhsT chunks of the current layer input ([P, xtpad] bf16)
            xt_cur = []
            for k in range(din0 // P):
                t = xt_pool.tile([P, cfg.xtpad], BF16, tag="xt", name=f"xt0_{k}")
                nc.sync.dma_start(out=t[:], in_=xT_d[k * P:(k + 1) * P, :])
                xt_cur.append(t)

            for l in range(nl):
                din, dout = cfg.dims[l]
                pad = cfg.pads[l]
                used = dout + 2               # h | ss | one
                kc = din // P
                last = l == nl - 1
                agi, ago, sdt = ag_in[l], ag_out[l], sdtab[l]

                w_tiles = []
                for k in range(kc):
                    t = w_pool.tile([P, maxpad], BF16, tag="w")
                    nc.sync.dma_start(out=t[:, 0:pad], in_=W_d[l][k * P:(k + 1) * P, :])
                    w_tiles.append(t)

                # ---- phase A ----
                for m in range(cfg.NT):
                    rows = P if m < cfg.NT - 1 else cfg.nlast
                    ph = acc_pool.tile([P, maxpad], F32, space="PSUM", tag="acc")
                    for k in range(kc):
                        for (a, b) in col_splits(used):
                            nc.tensor.matmul(
                                out=ph[:, a:b],
                                lhsT=xt_cur[k][:, m * P:(m + 1) * P],
                                rhs=w_tiles[k][:, a:b],
                                start=(k == 0),
                                stop=(k == kc - 1),
                            )
                    # sd broadcast -> local sd table (cols: dout=ss, dout+1=sd)
                    sdbc = s_pool.tile([P, P], BF16, tag="sdbc")
                    nc.vector.tensor_copy(
                        out=sdbc[:],
                        in_=ph[:, dout + 1:dout + 2].to_broadcast([P, P]))
                    nc.sync.dma_start(out=sdt[m * P:(m + 1) * P, :], in_=sdbc[:])
                    # staged row [h | ss | 1]
                    stg = stg_pool.tile([P, maxpad], BF16, tag="stg")
                    nc.vector.tensor_copy(out=stg[:, 0:dout + 1], in_=ph[:, 0:dout + 1])
                    nc.vector.memset(stg[:, dout + 1:pad], 1.0)
                    nc.sync.dma_start(
                        out=agi[m * P:m * P + rows, 0:pad],
                        in_=stg[0:rows, 0:pad],
                    )

                # ---- phase B: all-gather the table (also the barrier) ----
                nc.gpsimd.collective_compute(
                    "AllGather",
                    mybir.AluOpType.bypass,
                    replica_groups=[list(range(NCORES))],
                    ins=[agi[:, :].opt()],
                    outs=[ago[:, :].opt()],
                )

                # ---- phase C ----
                xt_next = []
                if not last:
                    for k in range(dout // P):
                        xt_next.append(xt_pool.tile([P, cfg.xtpad], BF16,
                                                    tag="xt", name=f"xtn{l}_{k}"))

                for m in range(cfg.NT):
                    rows = P if m < cfg.NT - 1 else cfg.nlast
                    # gathers are split into <=GP-chunk pieces: the SWDGE ring
                    # holds 128 in-flight descriptors per lane and one gather
                    # burns num_idxs/16+1, so whole-tile gathers deadlock.
                    GP = 6
                    g = g_pool.tile([P, cfg.C, pad], BF16, tag="g")
                    sdg = sdg_pool.tile([P, cfg.C, P], BF16, tag="sdg")
                    for p0 in range(0, cfg.C, GP):
                        pc = min(GP, cfg.C - p0)
                        nc.gpsimd.dma_gather(
                            out_ap=g[:, p0:p0 + pc, :],
                            in_ap=ago[:, :],
                            idxs_ap=hidx_sb[m][:, p0 * 8:(p0 + pc) * 8],
                            num_idxs=pc * P,
                            num_idxs_reg=pc * P,
                            elem_size=pad,
                            queue_num=self_qn[0] % 4,
                        )
                        self_qn[0] += 1
                        nc.gpsimd.dma_gather(
                            out_ap=sdg[:, p0:p0 + pc, :],
                            in_ap=sdt[:, :],
                            idxs_ap=sdidx_sb[m][:, p0 * 8:(p0 + pc) * 8],
                            num_idxs=pc * P,
                            num_idxs_reg=pc * P,
                            elem_size=P,
                            queue_num=self_qn[0] % 4,
                        )
                        self_qn[0] += 1

                    # e = ss_src + sd_dst ; w = exp(leakyrelu(e))
                    e_all = small_pool.tile([P, cfg.C], F32, tag="eall")
                    nc.vector.tensor_add(
                        out=e_all[:],
                        in0=g[:, :, dout:dout + 1].rearrange("p c o -> p (c o)"),
                        in1=sdg[:, :, 0:1].rearrange("p c o -> p (c o)"),
                    )
                    w_all = small_pool.tile([P, cfg.C], F32, tag="wall")
                    nc.vector.scalar_tensor_tensor(
                        out=w_all[:], in0=e_all[:], scalar=NEG_SLOPE,
                        in1=e_all[:],
                        op0=mybir.AluOpType.mult, op1=mybir.AluOpType.max,
                    )
                    nc.scalar.activation(
                        out=w_all[:], in_=w_all[:],
                        func=mybir.ActivationFunctionType.Exp,
                    )

                    po = acc_pool.tile([P, maxpad], F32, space="PSUM", tag="acc")
                    for j in range(cfg.C):
                        s = s_pool.tile([P, P], BF16, tag="s")
                        nc.vector.scalar_tensor_tensor(
                            out=s[:], in0=iota_sb[:],
                            scalar=dl_sb[m][:, j:j + 1],
                            in1=w_all[:, j:j + 1].to_broadcast([P, P]),
                            op0=mybir.AluOpType.is_equal,
                            op1=mybir.AluOpType.mult,
                        )
                        for (a, b) in col_splits(used):
                            nc.tensor.matmul(
                                out=po[:, a:b], lhsT=s[:],
                                rhs=g[:, j, a:b],
                                start=(j == 0), stop=(j == cfg.C - 1),
                            )

                    # normalize: rec = 1/(den+eps); den = po[:, dout+1]
                    dtmp = small_pool.tile([P, 1], F32, tag="dtmp")
                    nc.vector.tensor_scalar_add(
                        out=dtmp[:], in0=po[:, dout + 1:dout + 2], scalar1=EPS)
                    rec = small_pool.tile([P, 1], F32, tag="rec")
                    nc.vector.reciprocal(out=rec[:], in_=dtmp[:])

                    if not last:
                        relu_t = stg_pool.tile([P, maxpad], BF16, tag="stg")
                        nc.vector.tensor_scalar(
                            out=relu_t[:, 0:dout], in0=po[:, 0:dout],
                            scalar1=rec[:, 0:1], scalar2=0.0,
                            op0=mybir.AluOpType.mult, op1=mybir.AluOpType.max,
                        )
                        for k in range(dout // P):
                            ptt = tp_pool.tile([P, P], BF16, space="PSUM", tag="tp")
                            nc.tensor.transpose(
                                out=ptt[:], in_=relu_t[:, k * P:(k + 1) * P],
                                identity=identb[:],
                            )
                            nc.vector.tensor_copy(
                                out=xt_next[k][:, m * P:(m + 1) * P], in_=ptt[:])
                    else:
                        # log_softmax over features
                        t1 = stg_pool.tile([P, dlast], F32, tag="t1")
                        nc.vector.tensor_scalar_mul(
                            out=t1[:], in0=po[:, 0:dout], scalar1=rec[:, 0:1])
                        mx = small_pool.tile([P, 1], F32, tag="mx")
                        nc.vector.reduce_max(out=mx[:], in_=t1[:],
                                             axis=mybir.AxisListType.X)
                        nc.vector.tensor_scalar_sub(
                            out=t1[:], in0=t1[:], scalar1=mx[:, 0:1])
                        ex = small_pool.tile([P, dlast], F32, tag="ex")
                        sm = small_pool.tile([P, 1], F32, tag="sm")
                        nc.scalar.activation(
                            out=ex[:], in_=t1[:],
                            func=mybir.ActivationFunctionType.Exp,
                            accum_out=sm[:])
                        lg = small_pool.tile([P, 1], F32, tag="lg")
                        nc.scalar.activation(
                            out=lg[:], in_=sm[:],
                            func=mybir.ActivationFunctionType.Ln)
                        nc.vector.tensor_scalar_sub(
                            out=t1[:], in0=t1[:], scalar1=lg[:, 0:1])
                        nc.sync.dma_start(
                            out=out_d[m * P:m * P + rows, :],
                            in_=t1[0:rows, 0:dlast],
                        )

                xt_cur = xt_next

    nc.compile()
    return nc


def _wrap16(flat):
    """[NIDX] -> [128, NIDX//16] int16: idx i at [i%16, i//16], replicated x8."""
    a = np.asarray(flat, np.int16).reshape(-1, 16).T
    return np.tile(a, (8, 1))


def prep_host(x, edge_index, Ws, a_srcs, a_dsts, cfg: Cfg):
    """Build per-core input maps."""
    N = cfg.N
    nl = len(cfg.dims)
    src = np.concatenate([np.asarray(edge_index[0]), np.arange(N, dtype=np.int64)])
    dst = np.concatenate([np.asarray(edge_index[1]), np.arange(N, dtype=np.int64)])
    src = src.astype(np.int64)
    dst = dst.astype(np.int64)

    c_of = dst // cfg.NPC
    r = dst - c_of * cfg.NPC
    t_of = r // P
    q = r - t_of * P
    key = c_of * cfg.NT + t_of
    order = np.argsort(key, kind="stable")
    counts = np.bincount(key, minlength=NCORES * cfg.NT)
    Cneed = int(np.ceil(counts.max() / P))
    assert Cneed <= cfg.C, f"need C>={Cneed}, got {cfg.C}"

    # flat edge-slot arrays: slot i = (chunk i//128, partition i%128)
    NI = cfg.C * P
    hidx = np.zeros((NCORES, cfg.NT, NI), dtype=np.int64)
    sdidx = np.zeros((NCORES, cfg.NT, NI), dtype=np.int64)
    dl_a = np.full((NCORES, cfg.NT, P, cfg.C), -1.0, dtype=np.float32)
    starts = np.zeros(NCORES * cfg.NT + 1, dtype=np.int64)
    np.cumsum(counts, out=starts[1:])
    for gk in range(NCORES * cfg.NT):
        seg = order[starts[gk]:starts[gk + 1]]
        if len(seg) == 0:
            continue
        c, t = divmod(gk, cfg.NT)
        k = np.arange(len(seg))
        jj = k // P
        pp = k % P
        hidx[c, t, jj * P + pp] = src[seg]
        sdidx[c, t, jj * P + pp] = r[seg]            # dst local row in core
        dl_a[c, t, pp, jj] = q[seg]

    # augmented weights [W | W@a_src | W@a_dst], bf16, padded
    Waug = []
    for l in range(nl):
        W = np.asarray(Ws[l], dtype=np.float32)
        was = W @ np.asarray(a_srcs[l], dtype=np.float32)
        wad = W @ np.asarray(a_dsts[l], dtype=np.float32)
        A = np.zeros((W.shape[0], cfg.pads[l]), dtype=np.float32)
        A[:, :W.shape[1]] = W
        A[:, W.shape[1]] = was
        A[:, W.shape[1] + 1] = wad
        Waug.append(A.astype(BF))

    iota = np.tile(np.arange(P, dtype=np.float32), (P, 1))

    x = np.asarray(x, dtype=np.float32)
    in_maps = []
    for c in range(NCORES):
        xs = x[c * cfg.NPC:(c + 1) * cfg.NPC]          # [NPC, din0]
        xT = np.zeros((cfg.dims[0][0], cfg.xtpad), dtype=np.float32)
        xT[:, :cfg.NPC] = xs.T
        m = {
            "xT": np.ascontiguousarray(xT).astype(BF),
            "hidx": np.ascontiguousarray(
                np.stack([_wrap16(hidx[c, t]) for t in range(cfg.NT)])),
            "sdidx": np.ascontiguousarray(
                np.stack([_wrap16(sdidx[c, t]) for t in range(cfg.NT)])),
            "dl": np.ascontiguousarray(dl_a[c]),
            "iota": iota,
        }
        for l in range(nl):
            m[f"W{l}"] = Waug[l]
        in_maps.append(m)
    return in_maps


def run(x, edge_index, Ws, a_srcs, a_dsts, cfg: Cfg, trace=False):
    in_maps = prep_host(x, edge_index, Ws, a_srcs, a_dsts, cfg)
    nc = build_program(cfg)
    res = bass_utils.run_bass_kernel_spmd(
        nc, in_maps, core_ids=list(range(NCORES)), trace=trace)
    out = np.concatenate([res.results[c]["out"][:cfg.NPC] for c in range(NCORES)],
                         axis=0)
    return out, res


FULL_CFG_DIMS = [(256, 1024), (1024, 1024), (1024, 512), (512, 128)]


def _full_cfg(edge_index):
    N = 10000
    dst = np.concatenate([np.asarray(edge_index[1]), np.arange(N, dtype=np.int64)])
    npc = N // NCORES
    nt = (npc + P - 1) // P
    c_of = dst // npc
    r = dst - c_of * npc
    t_of = r // P
    counts = np.bincount(c_of * nt + t_of, minlength=NCORES * nt)
    C = int(np.ceil(counts.max() / P))
    return Cfg(N, FULL_CFG_DIMS, C)


def kernel(x, edge_index, W1, as1, ad1, b1, W2, as2, ad2, b2,
           W3, as3, ad3, b3, W4, as4, ad4, b4):
    for b in (b1, b2, b3, b4):
        assert not np.any(np.asarray(b)), "nonzero bias not implemented"
    cfg = _full_cfg(edge_index)
    out, _ = run(
        x, edge_index,
        [W1, W2, W3, W4], [as1, as2, as3, as4], [ad1, ad2, ad3, ad4], cfg)
    return out.astype(np.float32)
